# revision 10
# baseline (speedup 1.0000x reference)
"""Self-contained Trainium2 Bass kernel for nn_Attention_62560493633940.

Sharding: 16 heads split across 8 cores (2 q-heads + their shared kv-head
per core, tensor parallel); x / pos replicated; per-core partial output
projections (over that core's 128 o-columns) summed on host.

Math notes:
- pos_logits[h,q,k] = a[q,h] - a[k,h] + bh[h] with a = p @ Wh.T, so
  softmax_k(pos_logits) is independent of q (shift invariance) -> pos_attn
  is a rank-1 per-head key distribution; no [t,t,PF] diff tensor.  Both
  softmax row-sums are exactly 1, so the re-normalization in the reference
  is an identity and the gate mix is (1-g)*attn + g*pos_attn.
- The pos contribution to the output is a constant row per core:
  outRow = sum_hd gpos[hd] * Wo[hd,:].  It is returned as a separate tiny
  output and added on the host, so the device output is attention-only.
- AV (and optionally S) run as fp8e4 DoubleRow matmuls (2 contraction
  tiles per pass, 0.5 PE cycles/col).  exp() is emitted straight to fp8;
  some tiles use a Schraudolph-style exp (int8 bit trick) on DVE/Pool to
  offload the Act engine.
"""
import sys

if '/opt/trn_rl_repo' not in sys.path:
    sys.path.insert(0, '/opt/trn_rl_repo')

import numpy as np

import concourse.bass as bass
import concourse.bacc as bacc
import concourse.tile as tile
import concourse.mybir as mybir
from concourse import bass_utils

F32 = mybir.dt.float32
F16 = mybir.dt.float16
F8 = mybir.dt.float8e4
I8 = mybir.dt.int8

T = 1024      # sequence length
DIM = 1024    # model dim
H = 16        # heads
KVH = 4       # kv heads
HD = 64       # head dim
PD = 64       # pos dim
PF = 128      # pos feature dim
BASE = 10000.0
NC = 8        # cores

# config flags
QK8 = True            # fp8 DoubleRow for the S = q@k matmul
SCHRAUD_DVE = (6, 7)  # m-tiles whose exp runs on DVE (Schraudolph fp8)
SCHRAUD_POOL = ()     # unused: Pool cannot access PSUM

# wpack column layout (fp16, [128, _WCOLS])
_WQ = 0          # [128, 8*128] wq (k-major)
_WKV = 1024      # [128, 8*128]
_WO = 2048       # [128, 1024]
_TABC = 3072     # [128, 1024] cos table, 4x tiled rows
_TABS = 4096     # [128, 1024] [sin;-sin] 2x tiled rows
_WP1 = 5120      # [64, 64] (rows 64:128 zero)
_WP2 = 5184      # [64, 128]
_WH2 = 5312      # [128, 32]
_ID66 = 5344     # [66, 66] identity
_ONES = 5412     # [1, 128] ones row
_ONESW = 5540    # [1, 1024] ones row (wide)
_WCOLS = 6564

_SCH_A = float(8.0 / np.log(2.0) * 0.125)  # schraudolph scale (incl 1/8)
_SCH_B = 56.0

_CACHE = {}


def _build_program(reps=1):
    nc = bacc.Bacc("TRN2")

    wpack_d = nc.declare_dram_parameter("wpack", [128, _WCOLS], F16, isOutput=False)
    xT_d = nc.declare_dram_parameter("xT", [128, 8, T], F16, isOutput=False)
    posT_d = nc.declare_dram_parameter("posT", [PD, T], F16, isOutput=False)
    fpack_d = nc.declare_dram_parameter("fpack", [128, 6], F32, isOutput=False)
    outp_d = nc.declare_dram_parameter("outp", [T, DIM], F16, isOutput=True)
    outrow_d = nc.declare_dram_parameter("outrow", [1, DIM], F16, isOutput=True)

    ExpF = mybir.ActivationFunctionType.Exp
    ReluF = mybir.ActivationFunctionType.Relu
    IdentF = mybir.ActivationFunctionType.Identity
    SigF = mybir.ActivationFunctionType.Sigmoid
    MUL = mybir.AluOpType.mult
    ADD = mybir.AluOpType.add
    DR = mybir.MatmulPerfMode.DoubleRow

    with tile.TileContext(nc) as tc:
        with tc.tile_pool(name="cst", bufs=1) as cst, \
             tc.tile_pool(name="wk", bufs=1) as wk, \
             tc.tile_pool(name="eP", bufs=3) as eP, \
             tc.tile_pool(name="v8P", bufs=4) as v8P, \
             tc.tile_pool(name="v16P", bufs=8) as v16P, \
             tc.tile_pool(name="oS", bufs=3) as oS, \
             tc.tile_pool(name="psW", bufs=2, space="PSUM") as psW, \
             tc.tile_pool(name="psA", bufs=3, space="PSUM") as psA, \
             tc.tile_pool(name="psS", bufs=1, space="PSUM") as psS:

            for _rep in range(reps):
                # ---------------- input DMAs ----------------
                wp = cst.tile([128, _WCOLS], F16)
                nc.sync.dma_start(out=wp[:], in_=wpack_d.ap())
                xT_sb = cst.tile([128, 8, T], F16)
                xT_ap = xT_d.ap()
                nc.sync.dma_start(out=xT_sb[:, 0:2, :], in_=xT_ap[:, 0:2, :])
                nc.scalar.dma_start(out=xT_sb[:, 2:4, :], in_=xT_ap[:, 2:4, :])
                nc.sync.dma_start(out=xT_sb[:, 4:6, :], in_=xT_ap[:, 4:6, :])
                nc.gpsimd.dma_start(out=xT_sb[:, 6:8, :], in_=xT_ap[:, 6:8, :])
                posT_sb = cst.tile([PD, T], F16)
                nc.scalar.dma_start(out=posT_sb[:], in_=posT_d.ap())
                fp_sb = cst.tile([128, 6], F32)
                nc.scalar.dma_start(out=fp_sb[:], in_=fpack_d.ap())

                # handy views into wpack
                wo_v = wp[:, _WO:_WO + 1024]
                tabc_v = wp[:, _TABC:_TABC + 1024]
                tabs_v = wp[:, _TABS:_TABS + 1024]
                ones128_v = wp[0:1, _ONES:_ONES + 128]
                ones64_v = wp[0:1, _ONES:_ONES + 64]

                # host-precomputed sigmoid columns (keeps Act on one table)
                sgn_col = fp_sb[:, 3:4]
                sgp2 = fp_sb[0:2, 2:3]

                # ---------------- pos path (independent of x) ----------------
                pTr = wk.tile([PD, T], F16)
                for n in range(2):
                    pp = psS.tile([PD, 512], F32, tag="sm")
                    nc.tensor.matmul(pp[:], wp[0:64, _WP1:_WP1 + 64],
                                     posT_sb[:, 512 * n:512 * n + 512],
                                     start=True, stop=True)
                    nc.scalar.activation(pTr[:, 512 * n:512 * n + 512], pp[:],
                                         ReluF, bias=fp_sb[0:64, 1:2], scale=1.0)
                p2Tb = wk.tile([PF, T], F16)
                for n in range(2):
                    p2p = psS.tile([PF, 512], F32, tag="sm")
                    nc.tensor.matmul(p2p[:], wp[0:64, _WP2:_WP2 + 128],
                                     pTr[:, 512 * n:512 * n + 512],
                                     start=True, stop=True)
                    nc.scalar.activation(p2Tb[:, 512 * n:512 * n + 512], p2p[:],
                                         IdentF, bias=fp_sb[:, 0:1], scale=1.0)
                # ---------------- q / kv projections ----------------
                qraw = psW.tile([128, T], F32, tag="wide")
                kvraw = psW.tile([128, T], F32, tag="wide")
                for k in range(8):
                    for n in range(2):
                        nc.tensor.matmul(qraw[:, 512 * n:512 * n + 512],
                                         wp[:, _WQ + 128 * k:_WQ + 128 * k + 128],
                                         xT_sb[:, k, 512 * n:512 * n + 512],
                                         start=(k == 0), stop=(k == 7))
                        nc.tensor.matmul(kvraw[:, 512 * n:512 * n + 512],
                                         wp[:, _WKV + 128 * k:_WKV + 128 * k + 128],
                                         xT_sb[:, k, 512 * n:512 * n + 512],
                                         start=(k == 0), stop=(k == 7))

                # ---------------- v: 66-row augmented transpose ----------------
                # vT66 rows 0:64 = v.T, row 64 = ones (Z row), row 65 = ones
                vT66 = wk.tile([66, T], F16)
                nc.sync.dma_start(out=vT66[64:65, :], in_=wp[0:1, _ONESW:_ONESW + 1024])
                nc.sync.dma_start(out=vT66[65:66, :], in_=wp[0:1, _ONESW:_ONESW + 1024])
                nc.scalar.activation(vT66[0:64, 0:512], kvraw[64:128, 0:512],
                                     IdentF, scale=1.0)
                nc.scalar.activation(vT66[0:64, 512:1024], kvraw[64:128, 512:1024],
                                     IdentF, scale=1.0)
                va8 = []   # fp8 pair tiles for DoubleRow AV
                va16 = []  # fp16 tiles for posout
                for P in range(4):
                    va8.append(v8P.tile([128, 2, 80], F8, tag="va8",
                                        name=f"va8_{P}"))
                for m in range(8):
                    va16.append(v16P.tile([128, 66], F16, tag="va16",
                                          name=f"va16_{m}"))
                vtp4 = [psS.tile([128, 4, 66], F16, tag="sm", name="vtp4a"),
                        psA.tile([128, 4, 66], F16, tag="av", name="vtp4b")]
                for m in range(8):
                    vt = vtp4[m // 4]
                    nc.tensor.transpose(vt[:, m % 4, :],
                                        vT66[:, 128 * m:128 * m + 128],
                                        wp[0:66, _ID66:_ID66 + 66])
                    if m % 2 == 0:
                        nc.scalar.activation(va16[m][:], vt[:, m % 4, :],
                                             IdentF, scale=1.0)
                    else:
                        nc.vector.tensor_copy(va16[m][:], vt[:, m % 4, :])
                    nc.gpsimd.tensor_copy(va8[m // 2][:, m % 2, 0:66], va16[m][:])

                # pos logits + exp (after va16 so Act stays on one act table)
                eposAll = wk.tile([128, 256], F16)
                aALL = psS.tile([128, 256], F32, tag="sm")
                for j in range(8):
                    nc.tensor.matmul(aALL[:, 32 * j:32 * j + 32],
                                     p2Tb[:, 128 * j:128 * j + 128],
                                     wp[:, _WH2:_WH2 + 32], start=True, stop=True)
                nc.scalar.activation(eposAll[:], aALL[:], ExpF, scale=-1.0)

                # ---------------- RoPE on q ----------------
                T1 = wk.tile([128, T], F16)
                T2 = wk.tile([128, T], F16)
                T2s = wk.tile([128, T], F16)
                nc.vector.tensor_mul(T1[:], qraw[:], tabc_v)
                nc.vector.tensor_mul(T2[:], qraw[:], tabs_v)
                for b in range(4):
                    sr = (b // 2) * 64 + (1 - (b % 2)) * 32
                    ds = (b // 2) * 64 + (b % 2) * 32
                    eng = [nc.sync, nc.scalar, nc.sync, nc.scalar][b]
                    eng.dma_start(out=T2s[ds:ds + 32, :], in_=T2[sr:sr + 32, :])
                qTf = wk.tile([128, T], F8 if QK8 else F16)
                nc.gpsimd.tensor_add(qTf[:], T1[:], T2s[:])

                # ---------------- RoPE on k ----------------
                T1k = wk.tile([64, T], F16)
                T2k = wk.tile([64, T], F16)
                T2ks = wk.tile([64, T], F16)
                nc.vector.tensor_mul(T1k[:], kvraw[0:64, :], tabc_v[0:64, :])
                nc.vector.tensor_mul(T2k[:], kvraw[0:64, :], tabs_v[0:64, :])
                nc.scalar.dma_start(out=T2ks[0:32, :], in_=T2k[32:64, :])
                nc.sync.dma_start(out=T2ks[32:64, :], in_=T2k[0:32, :])
                if QK8:
                    kTf = wk.tile([64, T], F8)
                else:
                    kTf = wk.tile([128, T], F16)
                nc.gpsimd.tensor_add(kTf[0:64, :], T1k[:], T2ks[:])

                if QK8:
                    # fp8 DoubleRow layouts: [32, slot, T]
                    qT8 = wk.tile([32, 4, T], F8)
                    for b, eng in zip(range(4),
                                      [nc.sync, nc.scalar, nc.sync, nc.scalar]):
                        eng.dma_start(out=qT8[:, b, :], in_=qTf[32 * b:32 * b + 32, :])
                    kT8 = wk.tile([32, 2, T], F8)
                    nc.scalar.dma_start(out=kT8[:, 0, :], in_=kTf[0:32, :])
                    nc.sync.dma_start(out=kT8[:, 1, :], in_=kTf[32:64, :])
                else:
                    nc.sync.dma_start(out=kTf[64:128, :], in_=kTf[0:64, :])

                # ---------------- gpos -> constant output row ----------------
                posout = psA.tile([32, 66], F32, tag="av")
                for j in range(8):
                    nc.tensor.matmul(posout[:], eposAll[:, 32 * j:32 * j + 32],
                                     va16[j][:], start=(j == 0), stop=(j == 7))
                recipZp = wk.tile([2, 1], F32)
                nc.vector.reciprocal(recipZp[:], posout[0:2, 64:65])
                gz2 = wk.tile([2, 1], F32)
                nc.vector.tensor_mul(gz2[:], recipZp[:], sgp2)
                gpos2 = wk.tile([2, 64], F16)
                nc.vector.tensor_scalar_mul(gpos2[:], posout[0:2, 0:64], gz2[:, 0:1])
                gposTp = psA.tile([64, 2], F16, tag="av")
                nc.tensor.transpose(gposTp[:], gpos2[:], wp[0:2, _ID66:_ID66 + 2])
                gposT2 = wk.tile([128, 1], F16)
                nc.vector.tensor_copy(gposT2[0:64, :], gposTp[:, 0:1])
                nc.vector.tensor_copy(gposT2[64:128, :], gposTp[:, 1:2])
                # outRow = gposT2^T @ Wo (constant over t) -> host-side add
                outRow = wk.tile([1, 1024], F16)
                for n in range(2):
                    orp = psS.tile([1, 512], F32, tag="sm")
                    nc.tensor.matmul(orp[:], gposT2[:, 0:1],
                                     wo_v[:, 512 * n:512 * n + 512],
                                     start=True, stop=True)
                    nc.vector.tensor_copy(outRow[:, 512 * n:512 * n + 512],
                                          orp[:])
                nc.scalar.dma_start(out=outrow_d.ap(), in_=outRow[:])

                # ---------------- attention ----------------
                oT = wk.tile([128, T], F16)
                for i in range(2):
                    r = 64 * i
                    avh = [psA.tile([66, 512], F32, tag="av", name=f"avh{i}_0"),
                           psA.tile([66, 512], F32, tag="av", name=f"avh{i}_1")]
                    E8 = [None] * 4
                    for m in range(8):
                        S = psW.tile([128, T], F32, tag="wide")
                        if QK8:
                            for n in range(2):
                                nc.tensor.matmul(
                                    S[:, 512 * n:512 * n + 512],
                                    kT8[:, :, 128 * m:128 * m + 128],
                                    qT8[:, 2 * i:2 * i + 2, 512 * n:512 * n + 512],
                                    start=True, stop=True, perf_mode=DR)
                        else:
                            for n in range(2):
                                nc.tensor.matmul(
                                    S[:, 512 * n:512 * n + 512],
                                    kTf[r:r + 64, 128 * m:128 * m + 128],
                                    qTf[r:r + 64, 512 * n:512 * n + 512],
                                    start=True, stop=True)
                        if m % 2 == 0:
                            E8[m // 2] = eP.tile([128, 2, T], F8, tag="E",
                                                 name=f"E8_{i}_{m // 2}")
                        Eslot = E8[m // 2][:, m % 2, :]
                        if m in SCHRAUD_DVE:
                            nc.vector.tensor_scalar(Eslot.bitcast(I8), S[:],
                                                    _SCH_A, _SCH_B, MUL, ADD)
                        elif m in SCHRAUD_POOL:
                            nc.gpsimd.tensor_scalar(Eslot.bitcast(I8), S[:],
                                                    _SCH_A, _SCH_B, MUL, ADD)
                        else:
                            nc.scalar.activation(Eslot, S[:], ExpF, scale=0.125)
                        if m % 2 == 1:
                            for n in range(2):
                                nc.tensor.matmul(
                                    avh[n][:], va8[m // 2][:, :, 0:66],
                                    E8[m // 2][:, :, 512 * n:512 * n + 512],
                                    start=(m == 1), stop=(m == 7), perf_mode=DR)
                    for n in range(2):
                        c0 = 512 * n
                        rz = wk.tile([1, 512], F16, tag=f"rz{i}{n}")
                        with nc.allow_low_precision(reason="1/Z in f16 is fine vs fp8 E noise"):
                            nc.vector.reciprocal(rz[:], avh[n][64:65, :])
                        zbS = wk.tile([64, 512], F16, tag=f"zb{i}{n}")
                        nc.gpsimd.partition_broadcast(zbS[:], rz[:])
                        nc.vector.scalar_tensor_tensor(
                            oT[r:r + 64, c0:c0 + 512], avh[n][0:64, :],
                            fp_sb[0:64, 3 + i:4 + i], zbS[:], MUL, MUL)

                # ---------------- output projection ----------------
                outp_ap = outp_d.ap()
                ceng = [nc.scalar, nc.vector, nc.scalar, nc.vector,
                        nc.scalar, nc.vector, nc.scalar, nc.vector]
                deng = [nc.sync, nc.scalar, nc.sync, nc.scalar,
                        nc.sync, nc.scalar, nc.sync, nc.scalar]
                for j in range(8):
                    outS = oS.tile([128, DIM], F16, tag="outS")
                    po = psW.tile([128, DIM], F32, tag="wide")
                    for n in range(2):
                        nc.tensor.matmul(po[:, 512 * n:512 * n + 512],
                                         oT[:, 128 * j:128 * j + 128],
                                         wo_v[:, 512 * n:512 * n + 512],
                                         start=True, stop=True)
                    if ceng[j] is nc.scalar:
                        nc.scalar.activation(outS[:], po[:], IdentF, scale=1.0)
                    else:
                        ceng[j].tensor_copy(outS[:], po[:])
                    deng[j].dma_start(out=outp_ap[128 * j:128 * j + 128, :],
                                      in_=outS[:])

    nc.compile()
    return nc


def _host_inputs(inputs):
    """Per-core in_maps from the full inputs."""
    x = np.asarray(inputs["x"], np.float32)
    pos = np.asarray(inputs["pos"], np.float32)
    Wq = np.asarray(inputs["Wq"], np.float32)
    Wk = np.asarray(inputs["Wk"], np.float32)
    Wv = np.asarray(inputs["Wv"], np.float32)
    Wo = np.asarray(inputs["Wo"], np.float32)
    Wp1 = np.asarray(inputs["Wp1"], np.float32)
    bp1 = np.asarray(inputs["bp1"], np.float32)
    Wp2 = np.asarray(inputs["Wp2"], np.float32)
    bp2 = np.asarray(inputs["bp2"], np.float32)
    Wh = np.asarray(inputs["Wh"], np.float32)
    gate = np.asarray(inputs["gate"], np.float32)

    xT = np.ascontiguousarray(x[0].T).astype(np.float16)
    xT8 = xT.reshape(8, 128, T).transpose(1, 0, 2).copy()  # [128, 8, T]
    posT = np.ascontiguousarray(pos[0].T).astype(np.float16)

    # RoPE tables (transposed layout, tiled along partitions)
    j = np.arange(HD // 2, dtype=np.float32)
    theta = (BASE ** (-2.0 * j / HD)).astype(np.float32)
    freqs = np.arange(T, dtype=np.float32)[:, None] * theta  # [T, 32]
    cosT = np.cos(freqs).T.astype(np.float16)                # [32, T]
    sinT = np.sin(freqs).T.astype(np.float16)
    tabc = np.tile(cosT, (4, 1))                             # [128, T]
    tabs = np.tile(np.concatenate([sinT, -sinT], 0), (2, 1))  # [128, T]

    wp1T = np.zeros((128, PD), np.float16)
    wp1T[:PD] = Wp1.T.astype(np.float16)
    wp2T = np.zeros((128, PF), np.float16)
    wp2T[:PD] = Wp2.T.astype(np.float16)
    id66 = np.zeros((128, 66), np.float16)
    id66[:66, :66] = np.eye(66, dtype=np.float16)
    onesrow = np.zeros((128, 128), np.float16)
    onesrow[0, :] = 1.0
    oneswide = np.zeros((128, 1024), np.float16)
    oneswide[0, :] = 1.0

    in_maps = []
    for c in range(NC):
        g = c // 2
        wqb = Wq[128 * c:128 * c + 128, :]        # [128 out, 1024 in]
        wq_pack = wqb.T.reshape(8, 128, 128).transpose(1, 0, 2).reshape(128, 1024)
        wkvb = np.concatenate([Wk[64 * g:64 * g + 64, :],
                               Wv[64 * g:64 * g + 64, :]], 0)
        wkv_pack = wkvb.T.reshape(8, 128, 128).transpose(1, 0, 2).reshape(128, 1024)
        wo_c = Wo[:, 128 * c:128 * c + 128].T     # [128, 1024]
        whT2_c = np.zeros((128, 32), np.float16)
        whT2_c[:, 0:2] = Wh[2 * c:2 * c + 2, :].T.astype(np.float16)

        wpack = np.zeros((128, _WCOLS), np.float16)
        wpack[:, _WQ:_WQ + 1024] = wq_pack.astype(np.float16)
        wpack[:, _WKV:_WKV + 1024] = wkv_pack.astype(np.float16)
        wpack[:, _WO:_WO + 1024] = wo_c.astype(np.float16)
        wpack[:, _TABC:_TABC + 1024] = tabc
        wpack[:, _TABS:_TABS + 1024] = tabs
        wpack[:, _WP1:_WP1 + PD] = wp1T
        wpack[:, _WP2:_WP2 + PF] = wp2T
        wpack[:, _WH2:_WH2 + 32] = whT2_c
        wpack[:, _ID66:_ID66 + 66] = id66
        wpack[:, _ONES:_ONES + 128] = onesrow
        wpack[:, _ONESW:_ONESW + 1024] = oneswide

        def sig(v):
            return 1.0 / (1.0 + np.exp(-v))
        fpack = np.zeros((128, 6), np.float32)
        fpack[:, 0] = bp2
        fpack[:PD, 1] = bp1
        fpack[0, 2] = sig(gate[2 * c])
        fpack[1, 2] = sig(gate[2 * c + 1])
        fpack[:64, 3] = sig(-gate[2 * c])
        fpack[:64, 4] = sig(-gate[2 * c + 1])

        in_maps.append({
            "wpack": wpack, "xT": xT8, "posT": posT, "fpack": fpack,
        })
    return in_maps


def get_program(reps=1):
    key = f"nc{reps}"
    if key not in _CACHE:
        _CACHE[key] = _build_program(reps)
    return _CACHE[key]


def kernel(**inputs) -> np.ndarray:
    nc = get_program()
    in_maps = _host_inputs(inputs)
    res = bass_utils.run_bass_kernel_spmd(nc, in_maps, list(range(NC)))
    out = np.zeros((T, DIM), np.float32)
    for c in range(NC):
        out += res.results[c]["outp"].astype(np.float32)
        out += res.results[c]["outrow"].astype(np.float32)
    out += np.asarray(inputs["bo"], np.float32)
    return out.reshape(1, T, DIM)


# revision 14
# speedup vs baseline: 4.3094x; 4.3094x over previous
"""Self-contained Trainium2 Bass kernel for nn_Attention_62560493633940.

Sharding: 16 heads split across 8 cores (2 q-heads + their shared kv-head
per core, tensor parallel); x / pos replicated; per-core partial output
projections (over that core's 128 o-columns) summed on host.

Math notes:
- pos_logits[h,q,k] = a[q,h] - a[k,h] + bh[h] with a = p @ Wh.T, so
  softmax_k(pos_logits) is independent of q (shift invariance) -> pos_attn
  is a rank-1 per-head key distribution; no [t,t,PF] diff tensor.  Both
  softmax row-sums are exactly 1, so the re-normalization in the reference
  is an identity and the gate mix is (1-g)*attn + g*pos_attn.
- The pos contribution to the output is a constant row per core:
  outRow = sum_hd gpos[hd] * Wo[hd,:].  It is returned as a separate tiny
  output and added on the host, so the device output is attention-only.
- AV (and optionally S) run as fp8e4 DoubleRow matmuls (2 contraction
  tiles per pass, 0.5 PE cycles/col).  exp() is emitted straight to fp8;
  some tiles use a Schraudolph-style exp (int8 bit trick) on DVE/Pool to
  offload the Act engine.
"""
import sys

if '/opt/trn_rl_repo' not in sys.path:
    sys.path.insert(0, '/opt/trn_rl_repo')

import numpy as np

import concourse.bass as bass
import concourse.bacc as bacc
import concourse.tile as tile
import concourse.mybir as mybir
from concourse import bass_utils

F32 = mybir.dt.float32
F16 = mybir.dt.float16
F8 = mybir.dt.float8e4
I8 = mybir.dt.int8

T = 1024      # sequence length
DIM = 1024    # model dim
H = 16        # heads
KVH = 4       # kv heads
HD = 64       # head dim
PD = 64       # pos dim
PF = 128      # pos feature dim
BASE = 10000.0
NC = 8        # cores

# config flags
QK8 = True            # fp8 DoubleRow for the S = q@k matmul
USE_POOL = False      # use GpSimd for SBUF-only elementwise work
ZBS_ACT = True        # zbS copies on Act instead of DVE
OUTS_ACT = 3          # number of outS tiles evacuated by Act (rest DVE)
SCHRAUD_DVE = (6, 7)  # m-tiles whose exp runs on DVE (Schraudolph fp8)
SCHRAUD_POOL = ()     # unused: Pool cannot access PSUM

# wpack column layout (fp16, [128, _WCOLS])
_WQ = 0          # [128, 8*128] wq (k-major)
_WKV = 1024      # [128, 8*128]
_WO = 2048       # [128, 1024]
_TABC = 3072     # [128, 1024] cos table, 4x tiled rows
_TABS = 4096     # [128, 1024] [sin;-sin] 2x tiled rows
_WP1 = 5120      # [64, 64] (rows 64:128 zero)
_WP2 = 5184      # [64, 128]
_WH2 = 5312      # [128, 32]
_ID66 = 5344     # [66, 66] identity
_ONES = 5412     # [1, 128] ones row
_ONESW = 5540    # [1, 1024] ones row (wide)
_WCOLS = 6564

_SCH_A = float(8.0 / np.log(2.0) * 0.125)  # schraudolph scale (incl 1/8)
_SCH_B = 56.0

_CACHE = {}


def _build_program(reps=1):
    nc = bacc.Bacc("TRN2")

    wpack_d = nc.declare_dram_parameter("wpack", [128, _WCOLS], F16, isOutput=False)
    xT_d = nc.declare_dram_parameter("xT", [128, 8, T], F16, isOutput=False)
    posT_d = nc.declare_dram_parameter("posT", [PD, T], F16, isOutput=False)
    fpack_d = nc.declare_dram_parameter("fpack", [128, 6], F32, isOutput=False)
    outp_d = nc.declare_dram_parameter("outp", [T, DIM], F16, isOutput=True)
    outrow_d = nc.declare_dram_parameter("outrow", [128, 8], F16, isOutput=True)

    ExpF = mybir.ActivationFunctionType.Exp
    ReluF = mybir.ActivationFunctionType.Relu
    IdentF = mybir.ActivationFunctionType.Identity
    SigF = mybir.ActivationFunctionType.Sigmoid
    MUL = mybir.AluOpType.mult
    ADD = mybir.AluOpType.add
    DR = mybir.MatmulPerfMode.DoubleRow

    with tile.TileContext(nc) as tc:
        with tc.tile_pool(name="cst", bufs=2) as cst, \
             tc.tile_pool(name="wk", bufs=2) as wk, \
             tc.tile_pool(name="eP", bufs=4) as eP, \
             tc.tile_pool(name="v8P", bufs=8) as v8P, \
             tc.tile_pool(name="v16P", bufs=16) as v16P, \
             tc.tile_pool(name="oS", bufs=4) as oS, \
             tc.tile_pool(name="psW", bufs=2, space="PSUM") as psW, \
             tc.tile_pool(name="psA", bufs=3, space="PSUM") as psA, \
             tc.tile_pool(name="psS", bufs=1, space="PSUM") as psS:

            for _rep in range(reps):
                # ---------------- input DMAs ----------------
                wp = cst.tile([128, _WCOLS], F16)
                nc.sync.dma_start(out=wp[:], in_=wpack_d.ap())
                xT_sb = cst.tile([128, 8, T], F16)
                xT_ap = xT_d.ap()
                nc.sync.dma_start(out=xT_sb[:, 0:2, :], in_=xT_ap[:, 0:2, :])
                nc.scalar.dma_start(out=xT_sb[:, 2:4, :], in_=xT_ap[:, 2:4, :])
                nc.sync.dma_start(out=xT_sb[:, 4:6, :], in_=xT_ap[:, 4:6, :])
                (nc.gpsimd if USE_POOL else nc.scalar).dma_start(
                    out=xT_sb[:, 6:8, :], in_=xT_ap[:, 6:8, :])
                posT_sb = cst.tile([PD, T], F16)
                nc.scalar.dma_start(out=posT_sb[:], in_=posT_d.ap())
                fp_sb = cst.tile([128, 6], F32)
                nc.scalar.dma_start(out=fp_sb[:], in_=fpack_d.ap())

                # handy views into wpack
                wo_v = wp[:, _WO:_WO + 1024]
                tabc_v = wp[:, _TABC:_TABC + 1024]
                tabs_v = wp[:, _TABS:_TABS + 1024]
                ones128_v = wp[0:1, _ONES:_ONES + 128]
                ones64_v = wp[0:1, _ONES:_ONES + 64]

                # host-precomputed sigmoid columns (keeps Act on one table)
                sgn_col = fp_sb[:, 3:4]
                sgp2 = fp_sb[0:2, 2:3]

                # ---------------- pos path (independent of x) ----------------
                pTr = wk.tile([PD, T], F16)
                for n in range(2):
                    pp = psS.tile([PD, 512], F32, tag="sm")
                    nc.tensor.matmul(pp[:], wp[0:64, _WP1:_WP1 + 64],
                                     posT_sb[:, 512 * n:512 * n + 512],
                                     start=True, stop=True)
                    nc.scalar.activation(pTr[:, 512 * n:512 * n + 512], pp[:],
                                         ReluF, bias=fp_sb[0:64, 1:2], scale=1.0)
                p2Tb = wk.tile([PF, T], F16)
                for n in range(2):
                    p2p = psS.tile([PF, 512], F32, tag="sm")
                    nc.tensor.matmul(p2p[:], wp[0:64, _WP2:_WP2 + 128],
                                     pTr[:, 512 * n:512 * n + 512],
                                     start=True, stop=True)
                    nc.scalar.activation(p2Tb[:, 512 * n:512 * n + 512], p2p[:],
                                         IdentF, bias=fp_sb[:, 0:1], scale=1.0)
                # ---------------- q / kv projections ----------------
                qraw = psW.tile([128, T], F32, tag="wide")
                kvraw = psW.tile([128, T], F32, tag="wide")
                for k in range(8):
                    for n in range(2):
                        nc.tensor.matmul(qraw[:, 512 * n:512 * n + 512],
                                         wp[:, _WQ + 128 * k:_WQ + 128 * k + 128],
                                         xT_sb[:, k, 512 * n:512 * n + 512],
                                         start=(k == 0), stop=(k == 7))
                        nc.tensor.matmul(kvraw[:, 512 * n:512 * n + 512],
                                         wp[:, _WKV + 128 * k:_WKV + 128 * k + 128],
                                         xT_sb[:, k, 512 * n:512 * n + 512],
                                         start=(k == 0), stop=(k == 7))

                # ---------------- v: 66-row augmented transpose ----------------
                # vT66 rows 0:64 = v.T, row 64 = ones (Z row), row 65 = ones
                vT66 = wk.tile([66, T], F16)
                nc.sync.dma_start(out=vT66[64:65, :], in_=wp[0:1, _ONESW:_ONESW + 1024])
                nc.sync.dma_start(out=vT66[65:66, :], in_=wp[0:1, _ONESW:_ONESW + 1024])
                nc.scalar.activation(vT66[0:64, 0:512], kvraw[64:128, 0:512],
                                     IdentF, scale=1.0)
                nc.scalar.activation(vT66[0:64, 512:1024], kvraw[64:128, 512:1024],
                                     IdentF, scale=1.0)
                va8 = []   # fp8 pair tiles for DoubleRow AV
                va16 = []  # fp16 tiles for posout
                for P in range(4):
                    va8.append(v8P.tile([128, 2, 80], F8, tag="va8",
                                        name=f"va8_{P}"))
                for m in range(8):
                    va16.append(v16P.tile([128, 66], F16, tag="va16",
                                          name=f"va16_{m}"))
                vtp4 = [psS.tile([128, 4, 66], F16, tag="sm", name="vtp4a"),
                        psA.tile([128, 4, 66], F16, tag="av", name="vtp4b")]
                for m in range(8):
                    vt = vtp4[m // 4]
                    nc.tensor.transpose(vt[:, m % 4, :],
                                        vT66[:, 128 * m:128 * m + 128],
                                        wp[0:66, _ID66:_ID66 + 66])
                    if m % 2 == 0:
                        nc.scalar.activation(va16[m][:], vt[:, m % 4, :],
                                             IdentF, scale=1.0)
                    else:
                        nc.vector.tensor_copy(va16[m][:], vt[:, m % 4, :])
                    (nc.gpsimd if USE_POOL else nc.vector).tensor_copy(
                        va8[m // 2][:, m % 2, 0:66], va16[m][:])

                # pos logits + exp (after va16 so Act stays on one act table)
                eposAll = wk.tile([128, 256], F16)
                aALL = psS.tile([128, 256], F32, tag="sm")
                for j in range(8):
                    nc.tensor.matmul(aALL[:, 32 * j:32 * j + 32],
                                     p2Tb[:, 128 * j:128 * j + 128],
                                     wp[:, _WH2:_WH2 + 32], start=True, stop=True)
                nc.scalar.activation(eposAll[:], aALL[:], ExpF, scale=-1.0)

                # ---------------- RoPE on q ----------------
                T1 = wk.tile([128, T], F16)
                T2 = wk.tile([128, T], F16)
                T2s = wk.tile([128, T], F16)
                nc.vector.tensor_mul(T1[:], qraw[:], tabc_v)
                nc.vector.tensor_mul(T2[:], qraw[:], tabs_v)
                for b in range(4):
                    sr = (b // 2) * 64 + (1 - (b % 2)) * 32
                    ds = (b // 2) * 64 + (b % 2) * 32
                    eng = [nc.sync, nc.scalar, nc.sync, nc.scalar][b]
                    eng.dma_start(out=T2s[ds:ds + 32, :], in_=T2[sr:sr + 32, :])
                qTf = wk.tile([128, T], F8 if QK8 else F16)
                (nc.gpsimd if USE_POOL else nc.vector).tensor_add(
                    qTf[:], T1[:], T2s[:])

                # ---------------- RoPE on k ----------------
                T1k = wk.tile([64, T], F16)
                T2k = wk.tile([64, T], F16)
                T2ks = wk.tile([64, T], F16)
                nc.vector.tensor_mul(T1k[:], kvraw[0:64, :], tabc_v[0:64, :])
                nc.vector.tensor_mul(T2k[:], kvraw[0:64, :], tabs_v[0:64, :])
                nc.scalar.dma_start(out=T2ks[0:32, :], in_=T2k[32:64, :])
                nc.sync.dma_start(out=T2ks[32:64, :], in_=T2k[0:32, :])
                if QK8:
                    kTf = wk.tile([64, T], F8)
                else:
                    kTf = wk.tile([128, T], F16)
                (nc.gpsimd if USE_POOL else nc.vector).tensor_add(
                    kTf[0:64, :], T1k[:], T2ks[:])

                if QK8:
                    # fp8 DoubleRow layouts: [32, slot, T]
                    qT8 = wk.tile([32, 4, T], F8)
                    for b, eng in zip(range(4),
                                      [nc.sync, nc.scalar, nc.sync, nc.scalar]):
                        eng.dma_start(out=qT8[:, b, :], in_=qTf[32 * b:32 * b + 32, :])
                    kT8 = wk.tile([32, 2, T], F8)
                    nc.scalar.dma_start(out=kT8[:, 0, :], in_=kTf[0:32, :])
                    nc.sync.dma_start(out=kT8[:, 1, :], in_=kTf[32:64, :])
                else:
                    nc.sync.dma_start(out=kTf[64:128, :], in_=kTf[0:64, :])

                # ---------------- gpos -> constant output row ----------------
                posout = psA.tile([32, 66], F32, tag="av")
                for j in range(8):
                    nc.tensor.matmul(posout[:], eposAll[:, 32 * j:32 * j + 32],
                                     va16[j][:], start=(j == 0), stop=(j == 7))
                recipZp = wk.tile([2, 1], F32)
                nc.vector.reciprocal(recipZp[:], posout[0:2, 64:65])
                gz2 = wk.tile([2, 1], F32)
                nc.vector.tensor_mul(gz2[:], recipZp[:], sgp2)
                gpos2 = wk.tile([2, 64], F16)
                nc.vector.tensor_scalar_mul(gpos2[:], posout[0:2, 0:64], gz2[:, 0:1])
                gposTp = psA.tile([64, 2], F16, tag="av")
                nc.tensor.transpose(gposTp[:], gpos2[:], wp[0:2, _ID66:_ID66 + 2])
                gposT2 = wk.tile([128, 1], F16)
                nc.vector.tensor_copy(gposT2[0:64, :], gposTp[:, 0:1])
                nc.vector.tensor_copy(gposT2[64:128, :], gposTp[:, 1:2])
                # outRow = Wo^T @ gposT2 as a column per o-tile -> host-side add
                orp = psS.tile([128, 8], F32, tag="sm")
                for j in range(8):
                    nc.tensor.matmul(orp[:, j:j + 1],
                                     wo_v[:, 128 * j:128 * j + 128],
                                     gposT2[:, 0:1], start=True, stop=True)
                outRow = wk.tile([128, 8], F16)
                nc.vector.tensor_copy(outRow[:], orp[:])
                nc.scalar.dma_start(out=outrow_d.ap(), in_=outRow[:])

                # ---------------- attention ----------------
                oT = wk.tile([128, T], F16)
                for i in range(2):
                    r = 64 * i
                    avh = [psA.tile([66, 512], F32, tag="av", name=f"avh{i}_0"),
                           psA.tile([66, 512], F32, tag="av", name=f"avh{i}_1")]
                    E8 = [None] * 4
                    for m in range(8):
                        S = psW.tile([128, T], F32, tag="wide")
                        if QK8:
                            for n in range(2):
                                nc.tensor.matmul(
                                    S[:, 512 * n:512 * n + 512],
                                    kT8[:, :, 128 * m:128 * m + 128],
                                    qT8[:, 2 * i:2 * i + 2, 512 * n:512 * n + 512],
                                    start=True, stop=True, perf_mode=DR)
                        else:
                            for n in range(2):
                                nc.tensor.matmul(
                                    S[:, 512 * n:512 * n + 512],
                                    kTf[r:r + 64, 128 * m:128 * m + 128],
                                    qTf[r:r + 64, 512 * n:512 * n + 512],
                                    start=True, stop=True)
                        if m % 2 == 0:
                            E8[m // 2] = eP.tile([128, 2, T], F8, tag="E",
                                                 name=f"E8_{i}_{m // 2}")
                        Eslot = E8[m // 2][:, m % 2, :]
                        if m in SCHRAUD_DVE:
                            nc.vector.tensor_scalar(Eslot.bitcast(I8), S[:],
                                                    _SCH_A, _SCH_B, MUL, ADD)
                        elif m in SCHRAUD_POOL:
                            nc.gpsimd.tensor_scalar(Eslot.bitcast(I8), S[:],
                                                    _SCH_A, _SCH_B, MUL, ADD)
                        else:
                            nc.scalar.activation(Eslot, S[:], ExpF, scale=0.125)
                        if m % 2 == 1:
                            for n in range(2):
                                nc.tensor.matmul(
                                    avh[n][:], va8[m // 2][:, :, 0:66],
                                    E8[m // 2][:, :, 512 * n:512 * n + 512],
                                    start=(m == 1), stop=(m == 7), perf_mode=DR)
                    for n in range(2):
                        c0 = 512 * n
                        rz = wk.tile([1, 512], F16, tag=f"rz{i}{n}")
                        with nc.allow_low_precision(reason="1/Z in f16 is fine vs fp8 E noise"):
                            nc.vector.reciprocal(rz[:], avh[n][64:65, :])
                        zbS = wk.tile([64, 512], F16, tag=f"zb{i}{n}")
                        if USE_POOL:
                            nc.gpsimd.partition_broadcast(zbS[:], rz[:])
                        else:
                            zbp = psS.tile([64, 512], F32, tag="sm")
                            nc.tensor.matmul(zbp[:], ones64_v, rz[:],
                                             start=True, stop=True)
                            if ZBS_ACT:
                                nc.scalar.copy(zbS[:], zbp[:])
                            else:
                                nc.vector.tensor_copy(zbS[:], zbp[:])
                        nc.vector.scalar_tensor_tensor(
                            oT[r:r + 64, c0:c0 + 512], avh[n][0:64, :],
                            fp_sb[0:64, 3 + i:4 + i], zbS[:], MUL, MUL)

                # ---------------- output projection ----------------
                outp_ap = outp_d.ap()
                acts = [j * 8 // max(OUTS_ACT, 1) < 8 and
                        (j % (8 // max(OUTS_ACT, 1)) == 0 if OUTS_ACT else False)
                        for j in range(8)]
                nact = 0
                ceng = []
                for j in range(8):
                    if nact < OUTS_ACT and (j % 2 == 0 or 8 - j <= OUTS_ACT - nact):
                        ceng.append(nc.scalar); nact += 1
                    else:
                        ceng.append(nc.vector)
                deng = [nc.sync, nc.scalar, nc.sync, nc.scalar,
                        nc.sync, nc.scalar, nc.sync, nc.scalar]
                for j in range(8):
                    outS = oS.tile([128, DIM], F16, tag="outS")
                    po = psW.tile([128, DIM], F32, tag="wide")
                    for n in range(2):
                        nc.tensor.matmul(po[:, 512 * n:512 * n + 512],
                                         oT[:, 128 * j:128 * j + 128],
                                         wo_v[:, 512 * n:512 * n + 512],
                                         start=True, stop=True)
                    if ceng[j] is nc.scalar:
                        nc.scalar.activation(outS[:], po[:], IdentF, scale=1.0)
                    else:
                        ceng[j].tensor_copy(outS[:], po[:])
                    deng[j].dma_start(out=outp_ap[128 * j:128 * j + 128, :],
                                      in_=outS[:])

    nc.compile()
    return nc


def _host_inputs(inputs):
    """Per-core in_maps from the full inputs."""
    x = np.asarray(inputs["x"], np.float32)
    pos = np.asarray(inputs["pos"], np.float32)
    Wq = np.asarray(inputs["Wq"], np.float32)
    Wk = np.asarray(inputs["Wk"], np.float32)
    Wv = np.asarray(inputs["Wv"], np.float32)
    Wo = np.asarray(inputs["Wo"], np.float32)
    Wp1 = np.asarray(inputs["Wp1"], np.float32)
    bp1 = np.asarray(inputs["bp1"], np.float32)
    Wp2 = np.asarray(inputs["Wp2"], np.float32)
    bp2 = np.asarray(inputs["bp2"], np.float32)
    Wh = np.asarray(inputs["Wh"], np.float32)
    gate = np.asarray(inputs["gate"], np.float32)

    xT = np.ascontiguousarray(x[0].T).astype(np.float16)
    xT8 = xT.reshape(8, 128, T).transpose(1, 0, 2).copy()  # [128, 8, T]
    posT = np.ascontiguousarray(pos[0].T).astype(np.float16)

    # RoPE tables (transposed layout, tiled along partitions)
    j = np.arange(HD // 2, dtype=np.float32)
    theta = (BASE ** (-2.0 * j / HD)).astype(np.float32)
    freqs = np.arange(T, dtype=np.float32)[:, None] * theta  # [T, 32]
    cosT = np.cos(freqs).T.astype(np.float16)                # [32, T]
    sinT = np.sin(freqs).T.astype(np.float16)
    tabc = np.tile(cosT, (4, 1))                             # [128, T]
    tabs = np.tile(np.concatenate([sinT, -sinT], 0), (2, 1))  # [128, T]

    wp1T = np.zeros((128, PD), np.float16)
    wp1T[:PD] = Wp1.T.astype(np.float16)
    wp2T = np.zeros((128, PF), np.float16)
    wp2T[:PD] = Wp2.T.astype(np.float16)
    id66 = np.zeros((128, 66), np.float16)
    id66[:66, :66] = np.eye(66, dtype=np.float16)
    onesrow = np.zeros((128, 128), np.float16)
    onesrow[0, :] = 1.0
    oneswide = np.zeros((128, 1024), np.float16)
    oneswide[0, :] = 1.0

    in_maps = []
    for c in range(NC):
        g = c // 2
        wqb = Wq[128 * c:128 * c + 128, :]        # [128 out, 1024 in]
        wq_pack = wqb.T.reshape(8, 128, 128).transpose(1, 0, 2).reshape(128, 1024)
        wkvb = np.concatenate([Wk[64 * g:64 * g + 64, :],
                               Wv[64 * g:64 * g + 64, :]], 0)
        wkv_pack = wkvb.T.reshape(8, 128, 128).transpose(1, 0, 2).reshape(128, 1024)
        wo_c = Wo[:, 128 * c:128 * c + 128].T     # [128, 1024]
        whT2_c = np.zeros((128, 32), np.float16)
        whT2_c[:, 0:2] = Wh[2 * c:2 * c + 2, :].T.astype(np.float16)

        wpack = np.zeros((128, _WCOLS), np.float16)
        wpack[:, _WQ:_WQ + 1024] = wq_pack.astype(np.float16)
        wpack[:, _WKV:_WKV + 1024] = wkv_pack.astype(np.float16)
        wpack[:, _WO:_WO + 1024] = wo_c.astype(np.float16)
        wpack[:, _TABC:_TABC + 1024] = tabc
        wpack[:, _TABS:_TABS + 1024] = tabs
        wpack[:, _WP1:_WP1 + PD] = wp1T
        wpack[:, _WP2:_WP2 + PF] = wp2T
        wpack[:, _WH2:_WH2 + 32] = whT2_c
        wpack[:, _ID66:_ID66 + 66] = id66
        wpack[:, _ONES:_ONES + 128] = onesrow
        wpack[:, _ONESW:_ONESW + 1024] = oneswide

        def sig(v):
            return 1.0 / (1.0 + np.exp(-v))
        fpack = np.zeros((128, 6), np.float32)
        fpack[:, 0] = bp2
        fpack[:PD, 1] = bp1
        fpack[0, 2] = sig(gate[2 * c])
        fpack[1, 2] = sig(gate[2 * c + 1])
        fpack[:64, 3] = sig(-gate[2 * c])
        fpack[:64, 4] = sig(-gate[2 * c + 1])

        in_maps.append({
            "wpack": wpack, "xT": xT8, "posT": posT, "fpack": fpack,
        })
    return in_maps


def get_program(reps=1):
    key = f"nc{reps}"
    if key not in _CACHE:
        _CACHE[key] = _build_program(reps)
    return _CACHE[key]


def kernel(**inputs) -> np.ndarray:
    nc = get_program()
    in_maps = _host_inputs(inputs)
    res = bass_utils.run_bass_kernel_spmd(nc, in_maps, list(range(NC)))
    out = np.zeros((T, DIM), np.float32)
    for c in range(NC):
        out += res.results[c]["outp"].astype(np.float32)
        orow = res.results[c]["outrow"].astype(np.float32)
        out += orow.T.reshape(DIM)[None, :]
    out += np.asarray(inputs["bo"], np.float32)
    return out.reshape(1, T, DIM)
